# revision 3
# baseline (speedup 1.0000x reference)
"""Trainium2 Bass kernel for the DecoderCRF loss (B=64, S=512, D=512, T=12).

Math
----
reference loss = sum_b [ logZ_b - gold_b ] with feats = x @ W.T + b.

For the transitions matrix this problem ships (row START == -1e4, col
STOP == -1e4, everything else 0) and an all-ones mask, the forward
recursion collapses exactly (verified vs a float64 port of the reference):

    logZ_b  = sum_t log( sum_{j=0..9} exp(feats[b,t,j]) )
    gold_b  = sum_t feats[b,t,tags[b,t]]

feats entries are ~N(0,1) (range +-7), so exp() needs no max-shift in f32.

Device kernel (data-parallel over batch, 8 batch elements per core)
-------------------------------------------------------------------
Per core, x is [4096, 512] (rows = (b,s)).  Per 512-row group g:
  1. DMA x group -> SBUF [128, 4, 512]
  2. PE-transpose the 16 [128,128] blocks into xT_dc [128 d, 512 s] (d must
     sit on partitions for the feats matmul; DMA transpose is 2B-only)
  3. featsT = sum_dc Wt_dc.T @ xT_dc  -> PSUM [10, 512]   (tags 0..9 only)
  4. E = exp(featsT + bias)  (ScalarE, bias folded into the activation)
  5. colsum into row g of a persistent PSUM [8, 512] accumulator via a
     ones-selector matmul (partition-dim reduction done on PE)
  6. gold: accumulate onehot_tile.T @ x_tile into PSUM G [10, 512]
Tail: ln(colsums) + row-sum -> [8,1]; <G, W> row-dot -> [10,1]; DMA out.
Host sums the partials (and the trivial sum_t bias[tag] term) in f64.

Non-conforming inputs (different transitions pattern / mask / tag range)
fall back to a faithful numpy port of the reference.
"""

from contextlib import ExitStack

import numpy as np

N_CORES = 8
B, S, D = 64, 512, 512
T = 12
NT = 10          # tags that can actually appear / participate in the LSE
START, STOP = 10, 11
NEG = -10000.0
BS = B // N_CORES          # batch elements per core
R = BS * S                 # s-rows per core (4096)
N_GROUPS = 8               # 512-row groups per core
GROUP = R // N_GROUPS      # 512
F32 = None                 # set lazily (mybir import is heavy)

_NC_CACHE = None


def _build_nc():
    import concourse.bacc as bacc
    import concourse.mybir as mybir
    import concourse.tile as tile
    from concourse.masks import make_identity

    f32 = mybir.dt.float32
    nc = bacc.Bacc("TRN2", target_bir_lowering=False)

    x_d = nc.dram_tensor("x", [R, D], f32, kind="ExternalInput")
    wt_d = nc.dram_tensor("wt", [D, NT], f32, kind="ExternalInput")
    oh_d = nc.dram_tensor("oh", [R, NT], f32, kind="ExternalInput")
    gsel_d = nc.dram_tensor("gsel", [NT, 8 * N_GROUPS], f32, kind="ExternalInput")
    w10_d = nc.dram_tensor("w10", [NT, D], f32, kind="ExternalInput")
    b10_d = nc.dram_tensor("b10", [NT, 1], f32, kind="ExternalInput")
    out_d = nc.dram_tensor("out", [32, 1], f32, kind="ExternalOutput")

    x_r = x_d.rearrange("(g i p) d -> g p i d", g=N_GROUPS, p=128)      # [8,128,4,512]
    wt_r = wt_d.rearrange("(c p) m -> p c m", p=128)                    # [128,4,10]
    oh_r = oh_d.rearrange("(t p) m -> p t m", p=128)                    # [128,32,10]

    with tile.TileContext(nc) as tc, ExitStack() as ctx:
        consts = ctx.enter_context(tc.tile_pool(name="consts", bufs=1))
        xpool = ctx.enter_context(tc.tile_pool(name="xpool", bufs=3))
        xtpool = ctx.enter_context(tc.tile_pool(name="xtpool", bufs=10))
        epool = ctx.enter_context(tc.tile_pool(name="epool", bufs=2))
        fin = ctx.enter_context(tc.tile_pool(name="fin", bufs=1))
        pt = ctx.enter_context(tc.tile_pool(name="pt", bufs=4, space="PSUM"))
        pf = ctx.enter_context(tc.tile_pool(name="pf", bufs=2, space="PSUM"))
        pacc = ctx.enter_context(tc.tile_pool(name="pacc", bufs=1, space="PSUM"))

        ident = consts.tile([128, 128], f32)
        make_identity(nc, ident)
        wt_sb = consts.tile([128, 4, NT], f32)
        nc.sync.dma_start(out=wt_sb, in_=wt_r)
        oh_sb = consts.tile([128, R // 128, NT], f32)
        nc.sync.dma_start(out=oh_sb, in_=oh_r)
        gsel_sb = consts.tile([NT, 8 * N_GROUPS], f32)
        nc.sync.dma_start(out=gsel_sb, in_=gsel_d[:, :])
        w10_sb = consts.tile([NT, D], f32)
        nc.sync.dma_start(out=w10_sb, in_=w10_d[:, :])
        b10_sb = consts.tile([NT, 1], f32)
        nc.sync.dma_start(out=b10_sb, in_=b10_d[:, :])

        psum_c = pacc.tile([N_GROUPS, GROUP], f32, tag="csum")
        psum_g = pacc.tile([NT, D], f32, tag="gold")

        for g in range(N_GROUPS):
            x_sb = xpool.tile([128, 4, D], f32)
            nc.sync.dma_start(out=x_sb, in_=x_r[g])

            xts = []
            for dc in range(4):
                ptile = pt.tile([128, GROUP], f32)
                for i in range(4):
                    nc.tensor.transpose(
                        out=ptile[:, 128 * i : 128 * (i + 1)],
                        in_=x_sb[:, i, 128 * dc : 128 * (dc + 1)],
                        identity=ident,
                    )
                xt = xtpool.tile([128, GROUP], f32)
                if dc % 2 == 0:
                    nc.vector.tensor_copy(out=xt, in_=ptile)
                else:
                    nc.scalar.copy(out=xt, in_=ptile)
                xts.append(xt)

            psum_f = pf.tile([NT, GROUP], f32)
            for dc in range(4):
                nc.tensor.matmul(
                    psum_f,
                    lhsT=wt_sb[:, dc, :],
                    rhs=xts[dc],
                    start=(dc == 0),
                    stop=(dc == 3),
                )

            e_sb = epool.tile([NT, GROUP], f32)
            nc.scalar.activation(
                e_sb, psum_f, mybir.ActivationFunctionType.Exp, bias=b10_sb[:, :]
            )

            nc.tensor.matmul(
                psum_c,
                lhsT=gsel_sb[:, 8 * g : 8 * (g + 1)],
                rhs=e_sb,
                start=(g == 0),
                stop=(g == N_GROUPS - 1),
            )

            for i in range(4):
                nc.tensor.matmul(
                    psum_g,
                    lhsT=oh_sb[:, 4 * g + i, :],
                    rhs=x_sb[:, i, :],
                    start=(g == 0 and i == 0),
                    stop=(g == N_GROUPS - 1 and i == 3),
                )

        lnln = fin.tile([N_GROUPS, GROUP], f32)
        nc.scalar.activation(lnln, psum_c, mybir.ActivationFunctionType.Ln)
        lns = fin.tile([N_GROUPS, 1], f32)
        nc.vector.reduce_sum(out=lns, in_=lnln, axis=mybir.AxisListType.X)

        gw = fin.tile([NT, D], f32)
        nc.vector.tensor_mul(gw, psum_g, w10_sb)
        gdot = fin.tile([NT, 1], f32)
        nc.vector.reduce_sum(out=gdot, in_=gw, axis=mybir.AxisListType.X)

        nc.sync.dma_start(out=out_d[0:N_GROUPS, :], in_=lns)
        nc.sync.dma_start(out=out_d[N_GROUPS : N_GROUPS + NT, :], in_=gdot)

    nc.compile()
    return nc


def _get_nc():
    global _NC_CACHE
    if _NC_CACHE is None:
        _NC_CACHE = _build_nc()
    return _NC_CACHE


def _fast_path_ok(transitions, tags, mask):
    if transitions.shape != (T, T) or tags.min() < 0 or tags.max() >= NT:
        return False
    if not np.all(mask == 1):
        return False
    t2 = np.asarray(transitions, np.float64).copy()
    if not (np.all(t2[START, :] == NEG) and np.all(t2[:, STOP] == NEG)):
        return False
    t2[START, :] = 0.0
    t2[:, STOP] = 0.0
    return bool(np.all(t2 == 0.0))


def _reference_numpy(input_var, W, b, transitions, tags, mask):
    """Faithful float64 port of the reference (fallback only)."""
    x = np.asarray(input_var, np.float64)
    Wf = np.asarray(W, np.float64)
    bf = np.asarray(b, np.float64)
    tr = np.asarray(transitions, np.float64)
    mf = np.asarray(mask, np.float64)
    Bn, Sn, Dn = x.shape
    feats = (x.reshape(-1, Dn) @ Wf.T + bf).reshape(Bn, Sn, -1)
    fv = np.full((Bn, T), NEG)
    fv[:, START] = 0.0
    for t in range(Sn):
        tv = fv[:, None, :] + tr[None] + feats[:, t][:, :, None]
        m = tv.max(axis=2)
        new = m + np.log(np.exp(tv - m[:, :, None]).sum(axis=2))
        fv = new * mf[:, t : t + 1] + fv * (1 - mf[:, t : t + 1])
    fin = fv + tr[STOP][None]
    mm = fin.max(axis=1)
    alpha = mm + np.log(np.exp(fin - mm[:, None]).sum(axis=1))
    score0 = tr[tags[:, 0], START]
    emit = np.take_along_axis(feats[:, :-1], tags[:, :-1, None], axis=2)[..., 0]
    emit_sum = (emit * mf[:, :-1]).sum(axis=1)
    trs = tr[tags[:, 1:], tags[:, :-1]]
    trans_sum = (trs * mf[:, 1:]).sum(axis=1)
    last_idx = np.asarray(mask).sum(axis=1).astype(np.int64) - 1
    last_tags = np.take_along_axis(tags, last_idx[:, None], axis=1)[:, 0]
    last_emit = np.take_along_axis(feats[:, -1], last_tags[:, None], axis=1)[:, 0]
    gold = score0 + emit_sum + trans_sum + tr[STOP, last_tags] + last_emit * mf[:, -1]
    return np.float32((alpha - gold).sum())


def kernel(input_var, W, b, transitions, tags, mask):
    from concourse.bass_utils import run_bass_kernel_spmd

    input_var = np.asarray(input_var)
    W = np.asarray(W)
    b = np.asarray(b)
    transitions = np.asarray(transitions)
    tags = np.asarray(tags)
    mask = np.asarray(mask)

    if not _fast_path_ok(transitions, tags, mask):
        return _reference_numpy(input_var, W, b, transitions, tags, mask)

    nc = _get_nc()

    xf = np.ascontiguousarray(input_var.reshape(B * S, D), np.float32)
    wt = np.ascontiguousarray(W[:NT].T, np.float32)                 # [512, 10]
    w10 = np.ascontiguousarray(W[:NT], np.float32)                  # [10, 512]
    b10 = np.ascontiguousarray(b[:NT].reshape(NT, 1), np.float32)
    oh = np.zeros((B * S, NT), np.float32)
    oh[np.arange(B * S), tags.reshape(-1)] = 1.0
    gsel = np.zeros((NT, 8 * N_GROUPS), np.float32)
    for g in range(N_GROUPS):
        gsel[:, 8 * g + g] = 1.0

    in_maps = []
    for c in range(N_CORES):
        in_maps.append(
            {
                "x": xf[c * R : (c + 1) * R],
                "oh": oh[c * R : (c + 1) * R],
                "wt": wt,
                "w10": w10,
                "b10": b10,
                "gsel": gsel,
            }
        )

    res = run_bass_kernel_spmd(nc, in_maps, list(range(N_CORES)))

    total = np.float64(0.0)
    for c in range(N_CORES):
        o = np.asarray(res.results[c]["out"], np.float64)
        total += o[0:N_GROUPS, 0].sum() - o[N_GROUPS : N_GROUPS + NT, 0].sum()
    total -= np.asarray(b, np.float64)[tags].sum()   # gold bias term, host-side
    return np.float32(total)


# revision 5
# speedup vs baseline: 2.5818x; 2.5818x over previous
"""Trainium2 Bass kernel for the DecoderCRF loss (B=64, S=512, D=512, T=12).

Math
----
reference loss = sum_b [ logZ_b - gold_b ] with feats = x @ W.T + b.

For the transitions matrix this problem ships (row START == -1e4, col
STOP == -1e4, everything else 0) and an all-ones mask, the forward
recursion collapses exactly (verified vs a float64 port of the reference):

    logZ_b  = sum_t log( sum_{j=0..9} exp(feats[b,t,j]) )
    gold_b  = sum_t feats[b,t,tags[b,t]]

feats entries are ~N(0,1) (range +-7), so exp() needs no max-shift in f32.

Layout strategy (v2)
--------------------
f32 matmuls on the PE run 2-pass (fp32_mode=LOW_HIGH) and PE-side
transposes of x dominated v1 (PE 83us busy).  bf16 is numerically ample
here (loss abs err ~0.4 on 9.5e4), so the host ships x already
TRANSPOSED and CAST to bf16 — that removes every PE transpose, every
PSUM->SBUF copy, and halves the DMA bytes.  Data-parallel over batch:
core c handles batch elements 8c..8c+7.

Per core (xT [512 d, 4096 s] bf16, processed in 4 s-panels of 1024):
  1. featsT panel = sum_dc WtT_dc @ xT[dc, panel]    -> PSUM [10, 1024] f32
  2. E = exp(featsT + bias)  (ScalarE, bf16 out)
  3. per 512-half: ones-selector matmul accumulates col-sums of E into
     row g of a persistent PSUM [8, 512]  (partition reduction on PE)
  4. gold: DVE multiplies featsT PSUM by host-built onehotT panel
Tail: ln(colsums) + row-sum -> [8,1]; gold reduce -> [10,1]; DMA out.
Host sums partials (and the sum_t bias[tag] term) in f64.

Non-conforming inputs (different transitions pattern / mask / tag range)
fall back to a faithful numpy port of the reference.
"""

from contextlib import ExitStack

import numpy as np

N_CORES = 8
B, S, D = 64, 512, 512
T = 12
NT = 10          # tags that can actually appear / participate in the LSE
START, STOP = 10, 11
NEG = -10000.0
BS = B // N_CORES          # batch elements per core
R = BS * S                 # s-rows per core (4096)
N_GROUPS = 8               # 512-row groups per core (one batch element each)
GROUP = R // N_GROUPS      # 512
N_PANELS = 4               # s-panels per core
PANEL = R // N_PANELS      # 1024

_NC_CACHE = None


def _build_nc():
    import concourse.bacc as bacc
    import concourse.mybir as mybir
    import concourse.tile as tile

    f32 = mybir.dt.float32
    bf16 = mybir.dt.bfloat16
    nc = bacc.Bacc("TRN2", target_bir_lowering=False)

    xt_d = nc.dram_tensor("xt", [D, R], bf16, kind="ExternalInput")
    oht_d = nc.dram_tensor("oht", [NT, R], f32, kind="ExternalInput")
    wt_d = nc.dram_tensor("wt", [D, NT], bf16, kind="ExternalInput")
    gsel_d = nc.dram_tensor("gsel", [NT, 8 * N_GROUPS], bf16, kind="ExternalInput")
    b10_d = nc.dram_tensor("b10", [NT, 1], f32, kind="ExternalInput")
    out_d = nc.dram_tensor("out", [32, 1], f32, kind="ExternalOutput")

    xt_r = xt_d.rearrange("(c p) s -> p c s", p=128)   # [128, 4, 4096]
    wt_r = wt_d.rearrange("(c p) m -> p c m", p=128)   # [128, 4, 10]

    with tile.TileContext(nc) as tc, ExitStack() as ctx:
        consts = ctx.enter_context(tc.tile_pool(name="consts", bufs=1))
        xtp = ctx.enter_context(tc.tile_pool(name="xtp", bufs=3))
        epool = ctx.enter_context(tc.tile_pool(name="epool", bufs=2))
        fin = ctx.enter_context(tc.tile_pool(name="fin", bufs=1))
        pf = ctx.enter_context(tc.tile_pool(name="pf", bufs=2, space="PSUM"))
        pacc = ctx.enter_context(tc.tile_pool(name="pacc", bufs=1, space="PSUM"))

        wt_sb = consts.tile([128, 4, NT], bf16)
        nc.sync.dma_start(out=wt_sb, in_=wt_r)
        oht_sb = consts.tile([NT, R], f32)
        nc.sync.dma_start(out=oht_sb, in_=oht_d[:, :])
        gsel_sb = consts.tile([NT, 8 * N_GROUPS], bf16)
        nc.sync.dma_start(out=gsel_sb, in_=gsel_d[:, :])
        b10_sb = consts.tile([NT, 1], f32)
        nc.sync.dma_start(out=b10_sb, in_=b10_d[:, :])

        psum_c = pacc.tile([N_GROUPS, GROUP], f32, tag="csum")
        gw_all = fin.tile([NT, R], f32, tag="gw")

        for p in range(N_PANELS):
            xt_sb = xtp.tile([128, 4, PANEL], bf16)
            nc.sync.dma_start(out=xt_sb, in_=xt_r[:, :, p * PANEL : (p + 1) * PANEL])

            psum_f = pf.tile([NT, PANEL], f32)
            for dc in range(4):
                for h in range(2):      # matmul out must stay within one PSUM bank
                    nc.tensor.matmul(
                        psum_f[:, h * GROUP : (h + 1) * GROUP],
                        lhsT=wt_sb[:, dc, :],
                        rhs=xt_sb[:, dc, h * GROUP : (h + 1) * GROUP],
                        start=(dc == 0),
                        stop=(dc == 3),
                    )

            e_sb = epool.tile([NT, PANEL], bf16)
            nc.scalar.activation(
                e_sb, psum_f, mybir.ActivationFunctionType.Exp, bias=b10_sb[:, :]
            )

            for h in range(2):          # two 512-col groups per panel
                g = 2 * p + h
                nc.tensor.matmul(
                    psum_c,
                    lhsT=gsel_sb[:, 8 * g : 8 * (g + 1)],
                    rhs=e_sb[:, h * GROUP : (h + 1) * GROUP],
                    start=(g == 0),
                    stop=(g == 2 * N_PANELS - 1),
                )

            nc.vector.tensor_mul(
                gw_all[:, p * PANEL : (p + 1) * PANEL],
                psum_f,
                oht_sb[:, p * PANEL : (p + 1) * PANEL],
            )

        lnln = fin.tile([N_GROUPS, GROUP], f32)
        nc.scalar.activation(lnln, psum_c, mybir.ActivationFunctionType.Ln)
        lns = fin.tile([N_GROUPS, 1], f32)
        nc.vector.reduce_sum(out=lns, in_=lnln, axis=mybir.AxisListType.X)

        gdot = fin.tile([NT, 1], f32)
        nc.vector.reduce_sum(out=gdot, in_=gw_all, axis=mybir.AxisListType.X)

        nc.sync.dma_start(out=out_d[0:N_GROUPS, :], in_=lns)
        nc.sync.dma_start(out=out_d[N_GROUPS : N_GROUPS + NT, :], in_=gdot)

    nc.compile()
    return nc


def _get_nc():
    global _NC_CACHE
    if _NC_CACHE is None:
        _NC_CACHE = _build_nc()
    return _NC_CACHE


def _fast_path_ok(transitions, tags, mask):
    if transitions.shape != (T, T) or tags.min() < 0 or tags.max() >= NT:
        return False
    if not np.all(mask == 1):
        return False
    t2 = np.asarray(transitions, np.float64).copy()
    if not (np.all(t2[START, :] == NEG) and np.all(t2[:, STOP] == NEG)):
        return False
    t2[START, :] = 0.0
    t2[:, STOP] = 0.0
    return bool(np.all(t2 == 0.0))


def _reference_numpy(input_var, W, b, transitions, tags, mask):
    """Faithful float64 port of the reference (fallback only)."""
    x = np.asarray(input_var, np.float64)
    Wf = np.asarray(W, np.float64)
    bf = np.asarray(b, np.float64)
    tr = np.asarray(transitions, np.float64)
    mf = np.asarray(mask, np.float64)
    Bn, Sn, Dn = x.shape
    feats = (x.reshape(-1, Dn) @ Wf.T + bf).reshape(Bn, Sn, -1)
    fv = np.full((Bn, T), NEG)
    fv[:, START] = 0.0
    for t in range(Sn):
        tv = fv[:, None, :] + tr[None] + feats[:, t][:, :, None]
        m = tv.max(axis=2)
        new = m + np.log(np.exp(tv - m[:, :, None]).sum(axis=2))
        fv = new * mf[:, t : t + 1] + fv * (1 - mf[:, t : t + 1])
    fin = fv + tr[STOP][None]
    mm = fin.max(axis=1)
    alpha = mm + np.log(np.exp(fin - mm[:, None]).sum(axis=1))
    score0 = tr[tags[:, 0], START]
    emit = np.take_along_axis(feats[:, :-1], tags[:, :-1, None], axis=2)[..., 0]
    emit_sum = (emit * mf[:, :-1]).sum(axis=1)
    trs = tr[tags[:, 1:], tags[:, :-1]]
    trans_sum = (trs * mf[:, 1:]).sum(axis=1)
    last_idx = np.asarray(mask).sum(axis=1).astype(np.int64) - 1
    last_tags = np.take_along_axis(tags, last_idx[:, None], axis=1)[:, 0]
    last_emit = np.take_along_axis(feats[:, -1], last_tags[:, None], axis=1)[:, 0]
    gold = score0 + emit_sum + trans_sum + tr[STOP, last_tags] + last_emit * mf[:, -1]
    return np.float32((alpha - gold).sum())


def _make_in_maps(input_var, W, b, tags):
    import ml_dtypes

    bf16 = ml_dtypes.bfloat16
    wt = np.ascontiguousarray(W[:NT].T).astype(bf16)                # [512, 10]
    b10 = np.ascontiguousarray(b[:NT].reshape(NT, 1), np.float32)
    gsel = np.zeros((NT, 8 * N_GROUPS), np.float32)
    for g in range(N_GROUPS):
        gsel[:, 8 * g + g] = 1.0
    gsel = gsel.astype(bf16)

    xbf = input_var.reshape(B * S, D).astype(bf16)                  # one big cast
    onehot = np.zeros((B * S, NT), np.float32)
    onehot[np.arange(B * S), tags.reshape(-1)] = 1.0

    in_maps = []
    for c in range(N_CORES):
        xt = np.ascontiguousarray(xbf[c * R : (c + 1) * R].T)       # [512, 4096] bf16
        oht = np.ascontiguousarray(onehot[c * R : (c + 1) * R].T)   # [10, 4096] f32
        in_maps.append(
            {"xt": xt, "oht": oht, "wt": wt, "gsel": gsel, "b10": b10}
        )
    return in_maps


def kernel(input_var, W, b, transitions, tags, mask):
    from concourse.bass_utils import run_bass_kernel_spmd

    input_var = np.asarray(input_var)
    W = np.asarray(W)
    b = np.asarray(b)
    transitions = np.asarray(transitions)
    tags = np.asarray(tags)
    mask = np.asarray(mask)

    if not _fast_path_ok(transitions, tags, mask):
        return _reference_numpy(input_var, W, b, transitions, tags, mask)

    nc = _get_nc()
    in_maps = _make_in_maps(input_var, W, b, tags)
    res = run_bass_kernel_spmd(nc, in_maps, list(range(N_CORES)))

    total = np.float64(0.0)
    for c in range(N_CORES):
        o = np.asarray(res.results[c]["out"], np.float64)
        total += o[0:N_GROUPS, 0].sum() - o[N_GROUPS : N_GROUPS + NT, 0].sum()
    total -= np.asarray(b, np.float64)[tags].sum()   # gold bias term, host-side
    return np.float32(total)


# revision 8
# speedup vs baseline: 2.9132x; 1.1284x over previous
"""Trainium2 Bass kernel for the DecoderCRF loss (B=64, S=512, D=512, T=12).

Math
----
reference loss = sum_b [ logZ_b - gold_b ] with feats = x @ W.T + b.

For the transitions matrix this problem ships (row START == -1e4, col
STOP == -1e4, everything else 0) and an all-ones mask, the forward
recursion collapses exactly (verified vs a float64 port of the reference):

    logZ_b  = sum_t log( sum_{j=0..9} exp(feats[b,t,j]) )
    gold_b  = sum_t feats[b,t,tags[b,t]]

feats entries are ~N(0,1) (range +-7), so exp() needs no max-shift in f32.

Layout strategy (v2)
--------------------
f32 matmuls on the PE run 2-pass (fp32_mode=LOW_HIGH) and PE-side
transposes of x dominated v1 (PE 83us busy).  bf16 is numerically ample
here (loss abs err ~0.4 on 9.5e4), so the host ships x already
TRANSPOSED and CAST to bf16 — that removes every PE transpose, every
PSUM->SBUF copy, and halves the DMA bytes.  Data-parallel over batch:
core c handles batch elements 8c..8c+7.

Per core (xT [512 d, 4096 s] bf16, processed in 4 s-panels of 1024):
  1. featsT panel = sum_dc WtT_dc @ xT[dc, panel]    -> PSUM [10, 1024] f32
  2. E = exp(featsT + bias)  (ScalarE, bf16 out)
  3. per 512-half: ones-selector matmul accumulates col-sums of E into
     row g of a persistent PSUM [8, 512]  (partition reduction on PE)
  4. gold: DVE multiplies featsT PSUM by host-built onehotT panel
Tail: ln(colsums) + row-sum -> [8,1]; gold reduce -> [10,1]; DMA out.
Host sums partials (and the sum_t bias[tag] term) in f64.

Non-conforming inputs (different transitions pattern / mask / tag range)
fall back to a faithful numpy port of the reference.
"""

from contextlib import ExitStack

import numpy as np

N_CORES = 8
B, S, D = 64, 512, 512
T = 12
NT = 10          # tags that can actually appear / participate in the LSE
START, STOP = 10, 11
NEG = -10000.0
BS = B // N_CORES          # batch elements per core
R = BS * S                 # s-rows per core (4096)
N_GROUPS = 8               # 512-row groups per core (one batch element each)
GROUP = R // N_GROUPS      # 512
N_PANELS = 4               # s-panels per core
PANEL = R // N_PANELS      # 1024

_NC_CACHE = None


def _build_nc():
    import concourse.bacc as bacc
    import concourse.mybir as mybir
    import concourse.tile as tile

    f32 = mybir.dt.float32
    bf16 = mybir.dt.bfloat16
    nc = bacc.Bacc("TRN2", target_bir_lowering=False)

    xt_d = nc.dram_tensor("xt", [D, R], bf16, kind="ExternalInput")
    oht_d = nc.dram_tensor("oht", [NT, R], f32, kind="ExternalInput")
    wt_d = nc.dram_tensor("wt", [D, NT], bf16, kind="ExternalInput")
    gsel_d = nc.dram_tensor("gsel", [NT, 8 * N_GROUPS], bf16, kind="ExternalInput")
    b10_d = nc.dram_tensor("b10", [NT, 1], f32, kind="ExternalInput")
    out_d = nc.dram_tensor("out", [32, 1], f32, kind="ExternalOutput")

    xt_r = xt_d.rearrange("(c p) s -> p c s", p=128)   # [128, 4, 4096]
    wt_r = wt_d.rearrange("(c p) m -> p c m", p=128)   # [128, 4, 10]

    with tile.TileContext(nc) as tc, ExitStack() as ctx:
        consts = ctx.enter_context(tc.tile_pool(name="consts", bufs=1))
        xtp = ctx.enter_context(tc.tile_pool(name="xtp", bufs=3))
        epool = ctx.enter_context(tc.tile_pool(name="epool", bufs=2))
        fin = ctx.enter_context(tc.tile_pool(name="fin", bufs=1))
        pf = ctx.enter_context(tc.tile_pool(name="pf", bufs=2, space="PSUM"))
        pacc = ctx.enter_context(tc.tile_pool(name="pacc", bufs=1, space="PSUM"))

        wt_sb = consts.tile([128, 4, NT], bf16)
        nc.sync.dma_start(out=wt_sb, in_=wt_r)
        oht_sb = consts.tile([NT, R], f32)
        nc.gpsimd.dma_start(out=oht_sb, in_=oht_d[:, :])
        gsel_sb = consts.tile([NT, 8 * N_GROUPS], bf16)
        nc.sync.dma_start(out=gsel_sb, in_=gsel_d[:, :])
        b10_sb = consts.tile([NT, 1], f32)
        nc.sync.dma_start(out=b10_sb, in_=b10_d[:, :])

        psum_c = pacc.tile([N_GROUPS, GROUP], f32, tag="csum")
        gparts = fin.tile([NT, N_PANELS], f32, tag="gparts")

        for p in range(N_PANELS):
            # SWDGE: descriptor swizzle spreads the 1MB panel across all
            # 16 SDMA engines (HWDGE put everything on one engine)
            xt_sb = xtp.tile([128, 4, PANEL], bf16)
            nc.gpsimd.dma_start(out=xt_sb, in_=xt_r[:, :, p * PANEL : (p + 1) * PANEL])

            psum_f = pf.tile([NT, PANEL], f32)
            for dc in range(4):
                for h in range(2):      # matmul out must stay within one PSUM bank
                    nc.tensor.matmul(
                        psum_f[:, h * GROUP : (h + 1) * GROUP],
                        lhsT=wt_sb[:, dc, :],
                        rhs=xt_sb[:, dc, h * GROUP : (h + 1) * GROUP],
                        start=(dc == 0),
                        stop=(dc == 3),
                    )

            e_sb = epool.tile([NT, PANEL], bf16)
            nc.scalar.activation(
                e_sb, psum_f, mybir.ActivationFunctionType.Exp, bias=b10_sb[:, :]
            )

            for h in range(2):          # two 512-col groups per panel
                g = 2 * p + h
                nc.tensor.matmul(
                    psum_c,
                    lhsT=gsel_sb[:, 8 * g : 8 * (g + 1)],
                    rhs=e_sb[:, h * GROUP : (h + 1) * GROUP],
                    start=(g == 0),
                    stop=(g == 2 * N_PANELS - 1),
                )

            gw = xtp.tile([NT, PANEL], f32, tag="gw")
            nc.vector.tensor_mul(gw, psum_f, oht_sb[:, p * PANEL : (p + 1) * PANEL])
            nc.vector.reduce_sum(
                out=gparts[:, p : p + 1], in_=gw, axis=mybir.AxisListType.X
            )

        lnln = fin.tile([N_GROUPS, GROUP], f32)
        nc.scalar.activation(lnln, psum_c, mybir.ActivationFunctionType.Ln)
        lns = fin.tile([N_GROUPS, 1], f32)
        nc.vector.reduce_sum(out=lns, in_=lnln, axis=mybir.AxisListType.X)

        gdot = fin.tile([NT, 1], f32)
        nc.vector.reduce_sum(out=gdot, in_=gparts, axis=mybir.AxisListType.X)

        nc.sync.dma_start(out=out_d[0:N_GROUPS, :], in_=lns)
        nc.sync.dma_start(out=out_d[N_GROUPS : N_GROUPS + NT, :], in_=gdot)

    nc.compile()
    return nc


def _get_nc():
    global _NC_CACHE
    if _NC_CACHE is None:
        _NC_CACHE = _build_nc()
    return _NC_CACHE


def _fast_path_ok(transitions, tags, mask):
    if transitions.shape != (T, T) or tags.min() < 0 or tags.max() >= NT:
        return False
    if not np.all(mask == 1):
        return False
    t2 = np.asarray(transitions, np.float64).copy()
    if not (np.all(t2[START, :] == NEG) and np.all(t2[:, STOP] == NEG)):
        return False
    t2[START, :] = 0.0
    t2[:, STOP] = 0.0
    return bool(np.all(t2 == 0.0))


def _reference_numpy(input_var, W, b, transitions, tags, mask):
    """Faithful float64 port of the reference (fallback only)."""
    x = np.asarray(input_var, np.float64)
    Wf = np.asarray(W, np.float64)
    bf = np.asarray(b, np.float64)
    tr = np.asarray(transitions, np.float64)
    mf = np.asarray(mask, np.float64)
    Bn, Sn, Dn = x.shape
    feats = (x.reshape(-1, Dn) @ Wf.T + bf).reshape(Bn, Sn, -1)
    fv = np.full((Bn, T), NEG)
    fv[:, START] = 0.0
    for t in range(Sn):
        tv = fv[:, None, :] + tr[None] + feats[:, t][:, :, None]
        m = tv.max(axis=2)
        new = m + np.log(np.exp(tv - m[:, :, None]).sum(axis=2))
        fv = new * mf[:, t : t + 1] + fv * (1 - mf[:, t : t + 1])
    fin = fv + tr[STOP][None]
    mm = fin.max(axis=1)
    alpha = mm + np.log(np.exp(fin - mm[:, None]).sum(axis=1))
    score0 = tr[tags[:, 0], START]
    emit = np.take_along_axis(feats[:, :-1], tags[:, :-1, None], axis=2)[..., 0]
    emit_sum = (emit * mf[:, :-1]).sum(axis=1)
    trs = tr[tags[:, 1:], tags[:, :-1]]
    trans_sum = (trs * mf[:, 1:]).sum(axis=1)
    last_idx = np.asarray(mask).sum(axis=1).astype(np.int64) - 1
    last_tags = np.take_along_axis(tags, last_idx[:, None], axis=1)[:, 0]
    last_emit = np.take_along_axis(feats[:, -1], last_tags[:, None], axis=1)[:, 0]
    gold = score0 + emit_sum + trans_sum + tr[STOP, last_tags] + last_emit * mf[:, -1]
    return np.float32((alpha - gold).sum())


def _make_in_maps(input_var, W, b, tags):
    import ml_dtypes

    bf16 = ml_dtypes.bfloat16
    wt = np.ascontiguousarray(W[:NT].T).astype(bf16)                # [512, 10]
    b10 = np.ascontiguousarray(b[:NT].reshape(NT, 1), np.float32)
    gsel = np.zeros((NT, 8 * N_GROUPS), np.float32)
    for g in range(N_GROUPS):
        gsel[:, 8 * g + g] = 1.0
    gsel = gsel.astype(bf16)

    xbf = input_var.reshape(B * S, D).astype(bf16)                  # one big cast
    onehot = np.zeros((B * S, NT), np.float32)
    onehot[np.arange(B * S), tags.reshape(-1)] = 1.0

    in_maps = []
    for c in range(N_CORES):
        xt = np.ascontiguousarray(xbf[c * R : (c + 1) * R].T)       # [512, 4096] bf16
        oht = np.ascontiguousarray(onehot[c * R : (c + 1) * R].T)   # [10, 4096] f32
        in_maps.append(
            {"xt": xt, "oht": oht, "wt": wt, "gsel": gsel, "b10": b10}
        )
    return in_maps


def kernel(input_var, W, b, transitions, tags, mask):
    from concourse.bass_utils import run_bass_kernel_spmd

    input_var = np.asarray(input_var)
    W = np.asarray(W)
    b = np.asarray(b)
    transitions = np.asarray(transitions)
    tags = np.asarray(tags)
    mask = np.asarray(mask)

    if not _fast_path_ok(transitions, tags, mask):
        return _reference_numpy(input_var, W, b, transitions, tags, mask)

    nc = _get_nc()
    in_maps = _make_in_maps(input_var, W, b, tags)
    res = run_bass_kernel_spmd(nc, in_maps, list(range(N_CORES)))

    total = np.float64(0.0)
    for c in range(N_CORES):
        o = np.asarray(res.results[c]["out"], np.float64)
        total += o[0:N_GROUPS, 0].sum() - o[N_GROUPS : N_GROUPS + NT, 0].sum()
    total -= np.asarray(b, np.float64)[tags].sum()   # gold bias term, host-side
    return np.float32(total)
